# revision 16
# baseline (speedup 1.0000x reference)
"""Multi-head attention (b=4, n=2048, dim=1024, heads=16) on 8 TRN2 cores.

Sharding: tensor-parallel over heads (2 heads per core) + row-parallel output
projection; host sums the 8 partial outputs and adds the bias.

Fused single-phase schedule: the PE tensor engine drops to a 1.2 GHz p-state
whenever it idles and only reaches 2.4 GHz after ~3us of continuous work, so
the QKV projection (next batch) and the output projection (deferred) are
paced as filler matmuls inside the attention score/exp/attnV stream.  Per
et-step the PE does ~4 attention matmuls (852ns) plus enough filler to stay
just above ACT's exp cadence (~1.11us per [128,1024] tile), so the PE never
waits on ACT and holds max clock while ACT runs ~90% busy underneath.

Per-core math (heads h0=2c, h1=2c+1):
  qkv^T = w_in_c^T @ x^T            (bf16, w stationary, filler-paced)
  S^T   = k_h^T.T @ q_h^T           (bf16, K=64)
  E^T   = exp(S^T / 8)              (ACT, no max subtraction: scores ~ N(0,1))
  [o^T; denom] = [v_h | 1].T @ E^T  (bf16, M=65 fuses softmax denominator)
  o_norm^T = o^T * approx(1/denom)  (DVE reciprocal_approx_fast + bcast DMA)
  partial^T = w_out_c.T @ o_norm^T  (bf16, deferred to last-batch filler)
"""

import sys
import types
from collections import deque

import numpy as np

# NTFF-profile hook shim: container's antenv lacks axon_hooks; harmless if
# tracing is never requested.
if "antenv.axon_hooks" not in sys.modules:
    try:
        from trn_agent_boot.trn_boot import _ntff_profile_via_ctypes

        _m = types.ModuleType("antenv.axon_hooks")
        _h = _ntff_profile_via_ctypes("/opt/axon/libaxon_pjrt.so")
        _m.get_axon_ntff_profile_hook = lambda: _h
        _m.set_axon_ntff_profile_hook = lambda hook: None
        sys.modules["antenv.axon_hooks"] = _m
    except Exception:
        pass

import ml_dtypes

import concourse.bacc as bacc
import concourse.bass as bass
import concourse.mybir as mybir
import concourse.tile as tile
from concourse.bass_utils import run_bass_kernel_spmd

F32 = mybir.dt.float32
BF16 = mybir.dt.bfloat16

B, N, DIM, HEADS = 4, 2048, 1024, 16
HD = DIM // HEADS          # 64
NCORES = 8
HPC = HEADS // NCORES      # 2 heads per core
NT = B * N                 # 8192 tokens
MQKV = 3 * HPC * HD        # 384 qkv output dims per core
SCALE = HD ** -0.5         # 0.125

KT_TILES = DIM // 128      # 8 k-tiles in the projection contraction
BLK = 512                  # qkv filler token-block
NBLK = NT // BLK           # 16 blocks
BPB = N // BLK             # 4 blocks per batch
JT = N // 128              # 16 key j-tiles per batch
IH = N // 1024             # 2 query i-halves per batch


def _build_nc():
    nc = bacc.Bacc("TRN2", target_bir_lowering=False, debug=False)

    xT = nc.dram_tensor("xT", [DIM, NT], BF16, kind="ExternalInput")
    w_in_c = nc.dram_tensor("w_in_c", [DIM, MQKV], BF16, kind="ExternalInput")
    w_out_c = nc.dram_tensor("w_out_c", [128, DIM], BF16, kind="ExternalInput")
    po = nc.dram_tensor("po", [DIM, NT], BF16, kind="ExternalOutput")

    with tile.TileContext(nc) as tc:
        with (
            tc.tile_pool(name="big", bufs=1) as big,
            tc.tile_pool(name="strm", bufs=2) as strm,
            tc.tile_pool(name="et", bufs=6) as etp,
            tc.tile_pool(name="ps", bufs=1, space="PSUM") as ps,
        ):
            # ---- persistent SBUF ----
            QT = big.tile([128, NT], BF16)    # [q_h0(0:64); q_h1(64:128)]^T
            KT = big.tile([128, NT], BF16)
            # per j-tile: [v_h0 | 1 | v_h1 | 1]
            Vt = big.tile([128, B * JT, 130], BF16)
            o_sb = big.tile([128, NT], BF16)  # o^T both heads (normed in place)
            w_in_sb = big.tile([128, KT_TILES, MQKV], BF16)
            w_out_sb = big.tile([128, DIM], BF16)

            nc.vector.memset(Vt[:, :, 64], 1.0)
            nc.vector.memset(Vt[:, :, 129], 1.0)
            ones_sb = big.tile([1, 64], BF16)
            nc.vector.memset(ones_sb, 1.0)

            xT_r = xT.rearrange("(kt p) n -> p kt n", p=128)

            # ================= filler generators =================
            xin_tiles = {}
            dn_tiles = {}

            def xin_fetch(blk, split=False):
                if blk >= NBLK or blk in xin_tiles:
                    return
                t = strm.tile([128, KT_TILES, BLK], BF16, tag="xin", bufs=3)
                bsl = slice(blk * BLK, (blk + 1) * BLK)
                if split:
                    nc.sync.dma_start(out=t[:, 0:2, :], in_=xT_r[:, 0:2, bsl])
                    nc.sync.dma_start(out=t[:, 2:, :], in_=xT_r[:, 2:, bsl])
                else:
                    nc.sync.dma_start(out=t, in_=xT_r[:, :, bsl])
                xin_tiles[blk] = t

            def qkv_gen(b):
                """QKV projection for batch b: 4 blocks x 3 m-groups.
                Each yield emits <= 2 tensor-engine matmuls."""
                for blk4 in range(BPB):
                    blk = b * BPB + blk4
                    xin_fetch(blk)
                    xin = xin_tiles.pop(blk)
                    xin_fetch(blk + 1)
                    xin_fetch(blk + 2)
                    bcols = slice(blk * BLK, (blk + 1) * BLK)
                    for m in range(3):
                        pj = ps.tile(
                            [128, BLK], F32, tag="fill", bufs=2,
                            name=f"pj{blk}_{m}",
                        )
                        for k2 in range(KT_TILES // 2):
                            for k in (2 * k2, 2 * k2 + 1):
                                nc.tensor.matmul(
                                    pj,
                                    w_in_sb[:, k, m * 128:(m + 1) * 128],
                                    xin[:, k, :],
                                    start=(k == 0),
                                    stop=(k == KT_TILES - 1),
                                )
                            yield
                        if m == 0:
                            nc.vector.tensor_copy(QT[:, bcols], pj)
                        elif m == 1:
                            nc.vector.tensor_copy(KT[:, bcols], pj)
                        else:
                            vstage = strm.tile([128, BLK], BF16, tag="vstage")
                            nc.vector.tensor_copy(vstage, pj)
                            # XBAR transpose needs an offset-0 dense dest;
                            # one batched transpose per head ([64,512] ->
                            # [128,4,64] partition-tiles), then one gpsimd
                            # copy scatters it into Vt.
                            g0 = blk * (BLK // 128)
                            for hh in range(2):
                                ts = strm.tile(
                                    [128, BLK // 128, 64], BF16,
                                    tag="vt", bufs=4,
                                )
                                nc.sync.dma_start_transpose(
                                    out=ts,
                                    in_=vstage[hh * 64:(hh + 1) * 64, :],
                                )
                                nc.gpsimd.tensor_copy(
                                    Vt[
                                        :, g0:g0 + BLK // 128,
                                        hh * 65:hh * 65 + 64,
                                    ],
                                    ts,
                                )
                                if hh == 0:
                                    yield
                        yield

            def proj_gen(b, ihalf):
                """Output projection for one normalized 1024-token chunk.
                Each yield emits 1 tensor-engine matmul."""
                i0 = b * N + ihalf * 1024
                pending_dma = None
                for mt in range(DIM // 128):
                    pout = strm.tile([128, 1024], BF16, tag="pout", bufs=3)
                    for a in range(2):
                        pp = ps.tile(
                            [128, 512], F32, tag="fill", bufs=2,
                            name=f"pp{b}_{ihalf}_{mt}_{a}",
                        )
                        nc.tensor.matmul(
                            pp,
                            w_out_sb[:, mt * 128:(mt + 1) * 128],
                            o_sb[:, i0 + a * 512:i0 + (a + 1) * 512],
                            start=True, stop=True,
                        )
                        nc.vector.tensor_copy(
                            pout[:, a * 512:(a + 1) * 512], pp
                        )
                        if a == 1:
                            # defer the po write one yield so the Sync queue
                            # never parks on a DMA whose pout isn't cast yet
                            if pending_dma is not None:
                                nc.sync.dma_start(**pending_dma)
                            pending_dma = dict(
                                out=po[mt * 128:(mt + 1) * 128, i0:i0 + 1024],
                                in_=pout,
                            )
                        yield
                if pending_dma is not None:
                    nc.sync.dma_start(**pending_dma)

            class Pacer:
                """Drains filler generators evenly over a known step count."""

                def __init__(self):
                    self.gens = deque()
                    self.pending = 0

                def add(self, gen, n_yields):
                    self.gens.append(gen)
                    self.pending += n_yields

                def drain(self, k):
                    while k > 0 and self.gens:
                        try:
                            next(self.gens[0])
                        except StopIteration:
                            self.gens.popleft()
                            continue
                        self.pending -= 1
                        k -= 1

                def drain_paced(self, steps_left):
                    if not self.pending:
                        return
                    k = -(-self.pending // max(steps_left, 1))
                    self.drain(min(k, 4))

                def drain_all(self):
                    self.drain(self.pending + 8)

            QKV_YIELDS = BPB * 3 * (KT_TILES // 2 + 1)   # 60 per batch
            PROJ_YIELDS = (DIM // 128) * 2               # 16 per chunk

            # ================= attention chunks =================
            def chunk(b, ihalf, h, pacer, steps_left):
                i0 = b * N + ihalf * 1024
                icol = slice(i0, i0 + 1024)
                hp = slice(h * 64, (h + 1) * 64)
                seg = (b * IH + ihalf) * HPC + h
                po_t = ps.tile(
                    [128, 2, 512], F32, tag="po", name=f"po{seg}"
                )
                ets = {}
                for jt in range(JT + 1):
                    if jt < JT:
                        jcol = slice(b * N + jt * 128, b * N + jt * 128 + 128)
                        st = ps.tile(
                            [128, 2, 512], F32, tag="st", bufs=2,
                            name=f"st{seg}_{jt}",
                        )
                        for a in range(2):
                            nc.tensor.matmul(
                                st[:, a, :], KT[hp, jcol],
                                QT[hp, i0 + a * 512:i0 + (a + 1) * 512],
                                start=True, stop=True,
                            )
                        et = etp.tile([128, 1024], BF16, tag="et", name="et")
                        nc.scalar.activation(
                            et, st.rearrange("p a b -> p (a b)"),
                            mybir.ActivationFunctionType.Exp,
                            scale=SCALE,
                        )
                        ets[jt] = et
                    if jt > 0:
                        jp = jt - 1
                        et = ets.pop(jp)
                        for a in range(2):
                            nc.tensor.matmul(
                                po_t[0:65, a, :],
                                Vt[:, b * JT + jp, h * 65:h * 65 + 65],
                                et[:, a * 512:(a + 1) * 512],
                                start=(jp == 0), stop=(jp == JT - 1),
                            )
                    pacer.drain_paced(steps_left)
                    steps_left -= 1

                # ---- drain: denom row first (longer chain), then o ----
                po_o = po_t.rearrange("p a b -> p (a b)")
                dnst = strm.tile([1, 1024], F32, tag="dnst")
                nc.vector.tensor_copy(dnst, po_o[64:65, :])
                # reciprocal + bf16 cast now; broadcast happens at group end
                rcb = strm.tile([1, 1024], BF16, tag="rcb")
                with nc.allow_low_precision(reason="softmax denom approx"):
                    nc.vector.reciprocal_approx_fast(out=dnst, in_=dnst)
                    nc.vector.tensor_copy(rcb, dnst)
                dn_tiles[seg] = rcb
                if h == 0:
                    nc.vector.tensor_copy(o_sb[0:64, icol], po_o[0:64, :])
                else:
                    h1s = strm.tile([64, 1024], BF16, tag="h1s")
                    nc.vector.tensor_copy(h1s, po_o[0:64, :])
                    nc.sync.dma_start(out=o_sb[64:128, icol], in_=h1s)
                return steps_left

            def normalize(b, ihalf):
                # broadcast 1/denom to 64 partitions via a K=1 matmul with a
                # ones column (avoids the DRAM partition-broadcast bounce),
                # then multiply o_sb in place per (head, half).
                i0 = b * N + ihalf * 1024
                g0 = (b * IH + ihalf) * HPC
                rcb = [dn_tiles.pop(g0), dn_tiles.pop(g0 + 1)]
                for a in range(2):
                    bps = ps.tile(
                        [128, 512], F32, tag="fill", bufs=2,
                        name=f"bps{g0}_{a}",
                    )
                    asl = slice(a * 512, (a + 1) * 512)
                    for h in range(HPC):
                        nc.tensor.matmul(
                            bps[h * 64:(h + 1) * 64, :],
                            ones_sb[0:1, :],
                            rcb[h][0:1, asl],
                            start=True, stop=True,
                        )
                    for h in range(HPC):
                        rows = slice(h * 64, (h + 1) * 64)
                        icola = slice(i0 + a * 512, i0 + (a + 1) * 512)
                        nc.vector.tensor_mul(
                            o_sb[rows, icola], o_sb[rows, icola],
                            bps[rows, :],
                        )

            # ================= main schedule =================
            # Prologue: batch-0 QKV runs straight (PE ramps to max p-state).
            # x block 0 is the first DMA in the queue; weights follow.
            xin_fetch(0, split=True)
            nc.sync.dma_start(
                out=w_in_sb,
                in_=w_in_c.rearrange("(kt p) m -> p kt m", p=128),
            )
            pro = Pacer()
            pro.add(qkv_gen(0), QKV_YIELDS)
            pro.drain_all()
            nc.sync.dma_start(out=w_out_sb, in_=w_out_c[:, :])

            proj_ready = []
            for b in range(B):
                pacer = Pacer()
                if b < B - 1:
                    pacer.add(qkv_gen(b + 1), QKV_YIELDS)
                if b == B - 1:
                    # deferred output projections fill the last batch
                    for (pb, pih) in proj_ready:
                        pacer.add(proj_gen(pb, pih), PROJ_YIELDS)
                    proj_ready = []
                steps_left = IH * HPC * (JT + 1)   # 68 et-steps in this batch
                for ihalf in range(IH):
                    for h in range(HPC):
                        steps_left = chunk(b, ihalf, h, pacer, steps_left)
                    normalize(b, ihalf)
                    if b < B - 1:
                        proj_ready.append((b, ihalf))
                    else:
                        pacer.add(proj_gen(b, ihalf), PROJ_YIELDS)
                pacer.drain_all()

    nc.finalize()
    return nc


_CACHED = {}


def kernel(x, w_in, w_out, b_out, _trace=False):
    if "nc" not in _CACHED:
        _CACHED["nc"] = _build_nc()
    nc = _CACHED["nc"]

    x2 = np.ascontiguousarray(
        x.reshape(NT, DIM).T.astype(np.float32)
    )  # [DIM, NT]
    in_maps = []
    for c in range(NCORES):
        h0, h1 = HPC * c, HPC * c + 1
        cols = []
        for part in range(3):  # q, k, v
            base = part * DIM
            cols.extend(range(base + h0 * HD, base + h0 * HD + HD))
            cols.extend(range(base + h1 * HD, base + h1 * HD + HD))
        w_in_cc = np.ascontiguousarray(w_in[:, cols].astype(np.float32))
        w_out_cc = np.ascontiguousarray(
            w_out[128 * c:128 * (c + 1), :].astype(np.float32)
        )
        in_maps.append(
            {
                "xT": x2.astype(ml_dtypes.bfloat16),
                "w_in_c": w_in_cc.astype(ml_dtypes.bfloat16),
                "w_out_c": w_out_cc.astype(ml_dtypes.bfloat16),
            }
        )

    res = run_bass_kernel_spmd(
        nc, in_maps, core_ids=list(range(NCORES)), trace=_trace
    )
    acc = res.results[0]["po"].astype(np.float64)
    for c in range(1, NCORES):
        acc = acc + res.results[c]["po"].astype(np.float64)
    out = acc.T + b_out.astype(np.float64)
    if _trace:
        kernel.last_result = res
    return np.ascontiguousarray(out.reshape(B, N, DIM).astype(np.float32))


# revision 17
# speedup vs baseline: 1.0562x; 1.0562x over previous
"""Multi-head attention (b=4, n=2048, dim=1024, heads=16) on 8 TRN2 cores.

Sharding: tensor-parallel over heads (2 heads per core) + row-parallel output
projection; host sums the 8 partial outputs and adds the bias.

Fused single-phase schedule: the PE tensor engine drops to a 1.2 GHz p-state
whenever it idles and only reaches 2.4 GHz after ~3us of continuous work, so
the QKV projection (next batch) and the output projection (deferred) are
paced as filler matmuls inside the attention score/exp/attnV stream.  Per
et-step the PE does ~4 attention matmuls (852ns) plus enough filler to stay
just above ACT's exp cadence (~1.11us per [128,1024] tile), so the PE never
waits on ACT and holds max clock while ACT runs ~90% busy underneath.

Per-core math (heads h0=2c, h1=2c+1):
  qkv^T = w_in_c^T @ x^T            (bf16, w stationary, filler-paced)
  S^T   = k_h^T.T @ q_h^T           (bf16, K=64)
  E^T   = exp(S^T / 8)              (ACT, no max subtraction: scores ~ N(0,1))
  [o^T; denom] = [v_h | 1].T @ E^T  (bf16, M=65 fuses softmax denominator)
  o_norm^T = o^T * approx(1/denom)  (DVE reciprocal_approx_fast + bcast DMA)
  partial^T = w_out_c.T @ o_norm^T  (bf16, deferred to last-batch filler)
"""

import sys
import types
from collections import deque

import numpy as np

# NTFF-profile hook shim: container's antenv lacks axon_hooks; harmless if
# tracing is never requested.
if "antenv.axon_hooks" not in sys.modules:
    try:
        from trn_agent_boot.trn_boot import _ntff_profile_via_ctypes

        _m = types.ModuleType("antenv.axon_hooks")
        _h = _ntff_profile_via_ctypes("/opt/axon/libaxon_pjrt.so")
        _m.get_axon_ntff_profile_hook = lambda: _h
        _m.set_axon_ntff_profile_hook = lambda hook: None
        sys.modules["antenv.axon_hooks"] = _m
    except Exception:
        pass

import ml_dtypes

import concourse.bacc as bacc
import concourse.bass as bass
import concourse.mybir as mybir
import concourse.tile as tile
from concourse.bass_utils import run_bass_kernel_spmd

F32 = mybir.dt.float32
BF16 = mybir.dt.bfloat16

B, N, DIM, HEADS = 4, 2048, 1024, 16
HD = DIM // HEADS          # 64
NCORES = 8
HPC = HEADS // NCORES      # 2 heads per core
NT = B * N                 # 8192 tokens
MQKV = 3 * HPC * HD        # 384 qkv output dims per core
SCALE = HD ** -0.5         # 0.125

KT_TILES = DIM // 128      # 8 k-tiles in the projection contraction
BLK = 512                  # qkv filler token-block
NBLK = NT // BLK           # 16 blocks
BPB = N // BLK             # 4 blocks per batch
JT = N // 128              # 16 key j-tiles per batch
IH = N // 1024             # 2 query i-halves per batch


def _build_nc():
    nc = bacc.Bacc("TRN2", target_bir_lowering=False, debug=False)

    xT = nc.dram_tensor("xT", [DIM, NT], BF16, kind="ExternalInput")
    w_in_c = nc.dram_tensor("w_in_c", [DIM, MQKV], BF16, kind="ExternalInput")
    w_out_c = nc.dram_tensor("w_out_c", [128, DIM], BF16, kind="ExternalInput")
    po = nc.dram_tensor("po", [DIM, NT], BF16, kind="ExternalOutput")
    dn_dram = nc.dram_tensor("dn_dram", [16, 1024], F32)
    rc_dram = nc.dram_tensor("rc_dram", [16, 1024], F32)

    with tile.TileContext(nc) as tc:
        with (
            tc.tile_pool(name="big", bufs=1) as big,
            tc.tile_pool(name="strm", bufs=2) as strm,
            tc.tile_pool(name="et", bufs=6) as etp,
            tc.tile_pool(name="ps", bufs=1, space="PSUM") as ps,
        ):
            # ---- persistent SBUF ----
            QT = big.tile([128, NT], BF16)    # [q_h0(0:64); q_h1(64:128)]^T
            KT = big.tile([128, NT], BF16)
            # per j-tile: [v_h0 | 1 | v_h1 | 1]
            Vt = big.tile([128, B * JT, 130], BF16)
            o_sb = big.tile([128, NT], BF16)  # o^T both heads (normed in place)
            w_in_sb = big.tile([128, KT_TILES, MQKV], BF16)
            w_out_sb = big.tile([128, DIM], BF16)

            nc.vector.memset(Vt[:, :, 64], 1.0)
            nc.vector.memset(Vt[:, :, 129], 1.0)
            ones_sb = big.tile([1, 64], BF16)
            nc.vector.memset(ones_sb, 1.0)

            xT_r = xT.rearrange("(kt p) n -> p kt n", p=128)

            # ================= filler generators =================
            xin_tiles = {}
            dn_tiles = {}

            def xin_fetch(blk, split=False):
                if blk >= NBLK or blk in xin_tiles:
                    return
                t = strm.tile([128, KT_TILES, BLK], BF16, tag="xin", bufs=3)
                bsl = slice(blk * BLK, (blk + 1) * BLK)
                if split:
                    nc.sync.dma_start(out=t[:, 0:2, :], in_=xT_r[:, 0:2, bsl])
                    nc.sync.dma_start(out=t[:, 2:, :], in_=xT_r[:, 2:, bsl])
                else:
                    nc.sync.dma_start(out=t, in_=xT_r[:, :, bsl])
                xin_tiles[blk] = t

            def qkv_gen(b):
                """QKV projection for batch b: 4 blocks x 3 m-groups.
                Each yield emits <= 2 tensor-engine matmuls."""
                for blk4 in range(BPB):
                    blk = b * BPB + blk4
                    xin_fetch(blk)
                    xin = xin_tiles.pop(blk)
                    xin_fetch(blk + 1)
                    xin_fetch(blk + 2)
                    bcols = slice(blk * BLK, (blk + 1) * BLK)
                    for m in range(3):
                        pj = ps.tile(
                            [128, BLK], F32, tag="fill", bufs=2,
                            name=f"pj{blk}_{m}",
                        )
                        for k2 in range(KT_TILES // 2):
                            for k in (2 * k2, 2 * k2 + 1):
                                nc.tensor.matmul(
                                    pj,
                                    w_in_sb[:, k, m * 128:(m + 1) * 128],
                                    xin[:, k, :],
                                    start=(k == 0),
                                    stop=(k == KT_TILES - 1),
                                )
                            yield
                        if m == 0:
                            nc.vector.tensor_copy(QT[:, bcols], pj)
                        elif m == 1:
                            nc.vector.tensor_copy(KT[:, bcols], pj)
                        else:
                            vstage = strm.tile([128, BLK], BF16, tag="vstage")
                            nc.vector.tensor_copy(vstage, pj)
                            # XBAR transpose needs an offset-0 dense dest;
                            # one batched transpose per head ([64,512] ->
                            # [128,4,64] partition-tiles), then one gpsimd
                            # copy scatters it into Vt.
                            g0 = blk * (BLK // 128)
                            for hh in range(2):
                                ts = strm.tile(
                                    [128, BLK // 128, 64], BF16,
                                    tag="vt", bufs=4,
                                )
                                nc.sync.dma_start_transpose(
                                    out=ts,
                                    in_=vstage[hh * 64:(hh + 1) * 64, :],
                                )
                                nc.gpsimd.tensor_copy(
                                    Vt[
                                        :, g0:g0 + BLK // 128,
                                        hh * 65:hh * 65 + 64,
                                    ],
                                    ts,
                                )
                                if hh == 0:
                                    yield
                        yield

            def proj_gen(b, ihalf):
                """Output projection for one normalized 1024-token chunk.
                Each yield emits 1 tensor-engine matmul."""
                i0 = b * N + ihalf * 1024
                pending_dma = None
                for mt in range(DIM // 128):
                    pout = strm.tile([128, 1024], BF16, tag="pout", bufs=3)
                    for a in range(2):
                        pp = ps.tile(
                            [128, 512], F32, tag="fill", bufs=2,
                            name=f"pp{b}_{ihalf}_{mt}_{a}",
                        )
                        nc.tensor.matmul(
                            pp,
                            w_out_sb[:, mt * 128:(mt + 1) * 128],
                            o_sb[:, i0 + a * 512:i0 + (a + 1) * 512],
                            start=True, stop=True,
                        )
                        nc.vector.tensor_copy(
                            pout[:, a * 512:(a + 1) * 512], pp
                        )
                        if a == 1:
                            # defer the po write one yield so the Sync queue
                            # never parks on a DMA whose pout isn't cast yet
                            if pending_dma is not None:
                                nc.sync.dma_start(**pending_dma)
                            pending_dma = dict(
                                out=po[mt * 128:(mt + 1) * 128, i0:i0 + 1024],
                                in_=pout,
                            )
                        yield
                if pending_dma is not None:
                    nc.sync.dma_start(**pending_dma)

            class Pacer:
                """Drains filler generators evenly over a known step count."""

                def __init__(self):
                    self.gens = deque()
                    self.pending = 0

                def add(self, gen, n_yields):
                    self.gens.append(gen)
                    self.pending += n_yields

                def drain(self, k):
                    while k > 0 and self.gens:
                        try:
                            next(self.gens[0])
                        except StopIteration:
                            self.gens.popleft()
                            continue
                        self.pending -= 1
                        k -= 1

                def drain_paced(self, steps_left):
                    if not self.pending:
                        return
                    k = -(-self.pending // max(steps_left, 1))
                    self.drain(min(k, 4))

                def drain_all(self):
                    self.drain(self.pending + 8)

            QKV_YIELDS = BPB * 3 * (KT_TILES // 2 + 1)   # 60 per batch
            PROJ_YIELDS = (DIM // 128) * 2               # 16 per chunk

            # ================= attention chunks =================
            def chunk(b, ihalf, h, pacer, steps_left):
                i0 = b * N + ihalf * 1024
                icol = slice(i0, i0 + 1024)
                hp = slice(h * 64, (h + 1) * 64)
                seg = (b * IH + ihalf) * HPC + h
                po_t = ps.tile(
                    [128, 2, 512], F32, tag="po", name=f"po{seg}"
                )
                ets = {}
                for jt in range(JT + 1):
                    if jt < JT:
                        jcol = slice(b * N + jt * 128, b * N + jt * 128 + 128)
                        st = ps.tile(
                            [128, 2, 512], F32, tag="st", bufs=2,
                            name=f"st{seg}_{jt}",
                        )
                        for a in range(2):
                            nc.tensor.matmul(
                                st[:, a, :], KT[hp, jcol],
                                QT[hp, i0 + a * 512:i0 + (a + 1) * 512],
                                start=True, stop=True,
                            )
                        et = etp.tile([128, 1024], BF16, tag="et", name="et")
                        nc.scalar.activation(
                            et, st.rearrange("p a b -> p (a b)"),
                            mybir.ActivationFunctionType.Exp,
                            scale=SCALE,
                        )
                        ets[jt] = et
                    if jt > 0:
                        jp = jt - 1
                        et = ets.pop(jp)
                        for a in range(2):
                            nc.tensor.matmul(
                                po_t[0:65, a, :],
                                Vt[:, b * JT + jp, h * 65:h * 65 + 65],
                                et[:, a * 512:(a + 1) * 512],
                                start=(jp == 0), stop=(jp == JT - 1),
                            )
                    pacer.drain_paced(steps_left)
                    steps_left -= 1

                # ---- drain: denom row first (longer chain), then o ----
                po_o = po_t.rearrange("p a b -> p (a b)")
                dq = nc.scalar if seg == 15 else nc.sync
                dnst = strm.tile([1, 1024], F32, tag="dnst")
                nc.vector.tensor_copy(dnst, po_o[64:65, :])
                dq.dma_start(out=dn_dram[seg:seg + 1, :], in_=dnst[0:1, :])
                if h == 0:
                    nc.vector.tensor_copy(o_sb[0:64, icol], po_o[0:64, :])
                else:
                    h1s = strm.tile([64, 1024], BF16, tag="h1s")
                    nc.vector.tensor_copy(h1s, po_o[0:64, :])
                    nc.sync.dma_start(out=o_sb[64:128, icol], in_=h1s)
                return steps_left

            def normalize(b, ihalf):
                i0 = b * N + ihalf * 1024
                icol = slice(i0, i0 + 1024)
                g0 = (b * IH + ihalf) * HPC
                dq = nc.scalar if (b == B - 1 and ihalf == IH - 1) else nc.sync
                dns = strm.tile([2, 1024], F32, tag="dns")
                dq.dma_start(out=dns, in_=dn_dram[g0:g0 + 2, :])
                with nc.allow_low_precision(reason="softmax denom approx"):
                    nc.vector.reciprocal_approx_fast(out=dns, in_=dns)
                dq.dma_start(out=rc_dram[g0:g0 + 2, :], in_=dns)
                for h in range(HPC):
                    rows = slice(h * 64, (h + 1) * 64)
                    bcast = strm.tile([128, 1024], F32, tag="bcast")
                    src = rc_dram[g0 + h:g0 + h + 1, :]
                    rbc = bass.AP(
                        tensor=src.tensor,
                        offset=src.offset,
                        ap=[[0, 64]] + list(src.ap)[1:],
                    )
                    dq.dma_start(out=bcast[rows, :], in_=rbc)
                    eng = nc.gpsimd if h == 0 else nc.vector
                    eng.tensor_mul(
                        o_sb[rows, icol], o_sb[rows, icol], bcast[rows, :]
                    )

            # ================= main schedule =================
            # Prologue: batch-0 QKV runs straight (PE ramps to max p-state).
            # x block 0 is the first DMA in the queue; weights follow.
            xin_fetch(0, split=True)
            nc.sync.dma_start(
                out=w_in_sb,
                in_=w_in_c.rearrange("(kt p) m -> p kt m", p=128),
            )
            pro = Pacer()
            pro.add(qkv_gen(0), QKV_YIELDS)
            pro.drain_all()
            nc.sync.dma_start(out=w_out_sb, in_=w_out_c[:, :])

            proj_ready = []
            for b in range(B):
                pacer = Pacer()
                if b < B - 1:
                    pacer.add(qkv_gen(b + 1), QKV_YIELDS)
                if b == B - 1:
                    # deferred output projections fill the last batch
                    for (pb, pih) in proj_ready:
                        pacer.add(proj_gen(pb, pih), PROJ_YIELDS)
                    proj_ready = []
                steps_left = IH * HPC * (JT + 1)   # 68 et-steps in this batch
                for ihalf in range(IH):
                    for h in range(HPC):
                        steps_left = chunk(b, ihalf, h, pacer, steps_left)
                    normalize(b, ihalf)
                    if b < B - 1:
                        proj_ready.append((b, ihalf))
                    else:
                        pacer.add(proj_gen(b, ihalf), PROJ_YIELDS)
                pacer.drain_all()

    nc.finalize()
    return nc


_CACHED = {}


def kernel(x, w_in, w_out, b_out, _trace=False):
    if "nc" not in _CACHED:
        _CACHED["nc"] = _build_nc()
    nc = _CACHED["nc"]

    x2 = np.ascontiguousarray(
        x.reshape(NT, DIM).T.astype(np.float32)
    )  # [DIM, NT]
    in_maps = []
    for c in range(NCORES):
        h0, h1 = HPC * c, HPC * c + 1
        cols = []
        for part in range(3):  # q, k, v
            base = part * DIM
            cols.extend(range(base + h0 * HD, base + h0 * HD + HD))
            cols.extend(range(base + h1 * HD, base + h1 * HD + HD))
        w_in_cc = np.ascontiguousarray(w_in[:, cols].astype(np.float32))
        w_out_cc = np.ascontiguousarray(
            w_out[128 * c:128 * (c + 1), :].astype(np.float32)
        )
        in_maps.append(
            {
                "xT": x2.astype(ml_dtypes.bfloat16),
                "w_in_c": w_in_cc.astype(ml_dtypes.bfloat16),
                "w_out_c": w_out_cc.astype(ml_dtypes.bfloat16),
            }
        )

    res = run_bass_kernel_spmd(
        nc, in_maps, core_ids=list(range(NCORES)), trace=_trace
    )
    acc = res.results[0]["po"].astype(np.float64)
    for c in range(1, NCORES):
        acc = acc + res.results[c]["po"].astype(np.float64)
    out = acc.T + b_out.astype(np.float64)
    if _trace:
        kernel.last_result = res
    return np.ascontiguousarray(out.reshape(B, N, DIM).astype(np.float32))


# revision 18
# speedup vs baseline: 1.0697x; 1.0128x over previous
"""Multi-head attention (b=4, n=2048, dim=1024, heads=16) on 8 TRN2 cores.

Sharding: tensor-parallel over heads (2 heads per core) + row-parallel output
projection; host sums the 8 partial outputs and adds the bias.

Fused single-phase schedule: the PE tensor engine drops to a 1.2 GHz p-state
whenever it idles and only reaches 2.4 GHz after ~3us of continuous work, so
the QKV projection (next batch) and the output projection (deferred) are
paced as filler matmuls inside the attention score/exp/attnV stream.  Per
et-step the PE does ~4 attention matmuls (852ns) plus enough filler to stay
just above ACT's exp cadence (~1.11us per [128,1024] tile), so the PE never
waits on ACT and holds max clock while ACT runs ~90% busy underneath.

Per-core math (heads h0=2c, h1=2c+1):
  qkv^T = w_in_c^T @ x^T            (bf16, w stationary, filler-paced)
  S^T   = k_h^T.T @ q_h^T           (bf16, K=64)
  E^T   = exp(S^T / 8)              (ACT, no max subtraction: scores ~ N(0,1))
  [o^T; denom] = [v_h | 1].T @ E^T  (bf16, M=65 fuses softmax denominator)
  o_norm^T = o^T * approx(1/denom)  (DVE reciprocal_approx_fast + bcast DMA)
  partial^T = w_out_c.T @ o_norm^T  (bf16, deferred to last-batch filler)
"""

import sys
import types
from collections import deque

import numpy as np

# NTFF-profile hook shim: container's antenv lacks axon_hooks; harmless if
# tracing is never requested.
if "antenv.axon_hooks" not in sys.modules:
    try:
        from trn_agent_boot.trn_boot import _ntff_profile_via_ctypes

        _m = types.ModuleType("antenv.axon_hooks")
        _h = _ntff_profile_via_ctypes("/opt/axon/libaxon_pjrt.so")
        _m.get_axon_ntff_profile_hook = lambda: _h
        _m.set_axon_ntff_profile_hook = lambda hook: None
        sys.modules["antenv.axon_hooks"] = _m
    except Exception:
        pass

import ml_dtypes

import concourse.bacc as bacc
import concourse.bass as bass
import concourse.mybir as mybir
import concourse.tile as tile
from concourse.bass_utils import run_bass_kernel_spmd

F32 = mybir.dt.float32
BF16 = mybir.dt.bfloat16

B, N, DIM, HEADS = 4, 2048, 1024, 16
HD = DIM // HEADS          # 64
NCORES = 8
HPC = HEADS // NCORES      # 2 heads per core
NT = B * N                 # 8192 tokens
MQKV = 3 * HPC * HD        # 384 qkv output dims per core
SCALE = HD ** -0.5         # 0.125

KT_TILES = DIM // 128      # 8 k-tiles in the projection contraction
BLK = 512                  # qkv filler token-block
NBLK = NT // BLK           # 16 blocks
BPB = N // BLK             # 4 blocks per batch
JT = N // 128              # 16 key j-tiles per batch
IH = N // 1024             # 2 query i-halves per batch


def _build_nc():
    nc = bacc.Bacc("TRN2", target_bir_lowering=False, debug=False)

    xT = nc.dram_tensor("xT", [DIM, NT], BF16, kind="ExternalInput")
    w_in_c = nc.dram_tensor("w_in_c", [DIM, MQKV], BF16, kind="ExternalInput")
    w_out_c = nc.dram_tensor("w_out_c", [128, DIM], BF16, kind="ExternalInput")
    po = nc.dram_tensor("po", [DIM, NT], BF16, kind="ExternalOutput")
    dn_dram = nc.dram_tensor("dn_dram", [16, 1024], F32)
    rc_dram = nc.dram_tensor("rc_dram", [16, 1024], F32)

    with tile.TileContext(nc) as tc:
        with (
            tc.tile_pool(name="big", bufs=1) as big,
            tc.tile_pool(name="strm", bufs=2) as strm,
            tc.tile_pool(name="et", bufs=6) as etp,
            tc.tile_pool(name="ps", bufs=1, space="PSUM") as ps,
        ):
            # ---- persistent SBUF ----
            QT = big.tile([128, NT], BF16)    # [q_h0(0:64); q_h1(64:128)]^T
            KT = big.tile([128, NT], BF16)
            # per j-tile: [v_h0 | 1 | v_h1 | 1]
            Vt = big.tile([128, B * JT, 130], BF16)
            o_sb = big.tile([128, NT], BF16)  # o^T both heads (normed in place)
            w_in_sb = big.tile([128, KT_TILES, MQKV], BF16)
            w_out_sb = big.tile([128, DIM], BF16)

            nc.vector.memset(Vt[:, :, 64], 1.0)
            nc.vector.memset(Vt[:, :, 129], 1.0)
            ones_sb = big.tile([1, 64], BF16)
            nc.vector.memset(ones_sb, 1.0)

            xT_r = xT.rearrange("(kt p) n -> p kt n", p=128)

            # ================= filler generators =================
            xin_tiles = {}
            dn_tiles = {}

            def xin_fetch(blk, split=False):
                if blk >= NBLK or blk in xin_tiles:
                    return
                t = strm.tile([128, KT_TILES, BLK], BF16, tag="xin", bufs=3)
                bsl = slice(blk * BLK, (blk + 1) * BLK)
                if split:
                    nc.sync.dma_start(out=t[:, 0:2, :], in_=xT_r[:, 0:2, bsl])
                    nc.sync.dma_start(out=t[:, 2:, :], in_=xT_r[:, 2:, bsl])
                else:
                    nc.sync.dma_start(out=t, in_=xT_r[:, :, bsl])
                xin_tiles[blk] = t

            def qkv_gen(b):
                """QKV projection for batch b: 4 blocks x 3 m-groups.
                Each yield emits <= 2 tensor-engine matmuls."""
                for blk4 in range(BPB):
                    blk = b * BPB + blk4
                    xin_fetch(blk)
                    xin = xin_tiles.pop(blk)
                    xin_fetch(blk + 1)
                    xin_fetch(blk + 2)
                    bcols = slice(blk * BLK, (blk + 1) * BLK)
                    for m in range(3):
                        pj = ps.tile(
                            [128, BLK], F32, tag="fill", bufs=2,
                            name=f"pj{blk}_{m}",
                        )
                        for k2 in range(KT_TILES // 2):
                            for k in (2 * k2, 2 * k2 + 1):
                                nc.tensor.matmul(
                                    pj,
                                    w_in_sb[:, k, m * 128:(m + 1) * 128],
                                    xin[:, k, :],
                                    start=(k == 0),
                                    stop=(k == KT_TILES - 1),
                                )
                            yield
                        if m == 0:
                            nc.vector.tensor_copy(QT[:, bcols], pj)
                        elif m == 1:
                            nc.vector.tensor_copy(KT[:, bcols], pj)
                        else:
                            vstage = strm.tile([128, BLK], BF16, tag="vstage")
                            nc.vector.tensor_copy(vstage, pj)
                            # XBAR transpose needs an offset-0 dense dest;
                            # one batched transpose per head ([64,512] ->
                            # [128,4,64] partition-tiles), then one gpsimd
                            # copy scatters it into Vt.
                            g0 = blk * (BLK // 128)
                            for hh in range(2):
                                ts = strm.tile(
                                    [128, BLK // 128, 64], BF16,
                                    tag="vt", bufs=4,
                                )
                                nc.sync.dma_start_transpose(
                                    out=ts,
                                    in_=vstage[hh * 64:(hh + 1) * 64, :],
                                )
                                nc.gpsimd.tensor_copy(
                                    Vt[
                                        :, g0:g0 + BLK // 128,
                                        hh * 65:hh * 65 + 64,
                                    ],
                                    ts,
                                )
                                if hh == 0:
                                    yield
                        yield

            def proj_gen(b, ihalf):
                """Output projection for one normalized 1024-token chunk.
                Each yield emits 1 tensor-engine matmul."""
                i0 = b * N + ihalf * 1024
                pending_dma = None
                for mt in range(DIM // 128):
                    pout = strm.tile([128, 1024], BF16, tag="pout", bufs=3)
                    for a in range(2):
                        pp = ps.tile(
                            [128, 512], F32, tag="fill", bufs=2,
                            name=f"pp{b}_{ihalf}_{mt}_{a}",
                        )
                        nc.tensor.matmul(
                            pp,
                            w_out_sb[:, mt * 128:(mt + 1) * 128],
                            o_sb[:, i0 + a * 512:i0 + (a + 1) * 512],
                            start=True, stop=True,
                        )
                        nc.vector.tensor_copy(
                            pout[:, a * 512:(a + 1) * 512], pp
                        )
                        if a == 1:
                            # defer the po write one yield so the Sync queue
                            # never parks on a DMA whose pout isn't cast yet
                            if pending_dma is not None:
                                nc.sync.dma_start(**pending_dma)
                            pending_dma = dict(
                                out=po[mt * 128:(mt + 1) * 128, i0:i0 + 1024],
                                in_=pout,
                            )
                        yield
                if pending_dma is not None:
                    nc.sync.dma_start(**pending_dma)

            class Pacer:
                """Drains filler generators evenly over a known step count."""

                def __init__(self):
                    self.gens = deque()
                    self.pending = 0

                def add(self, gen, n_yields):
                    self.gens.append(gen)
                    self.pending += n_yields

                def drain(self, k):
                    while k > 0 and self.gens:
                        try:
                            next(self.gens[0])
                        except StopIteration:
                            self.gens.popleft()
                            continue
                        self.pending -= 1
                        k -= 1

                def drain_paced(self, steps_left):
                    if not self.pending:
                        return
                    k = -(-self.pending // max(steps_left, 1))
                    self.drain(min(k, 4))

                def drain_all(self):
                    self.drain(self.pending + 8)

            QKV_YIELDS = BPB * 3 * (KT_TILES // 2 + 1)   # 60 per batch
            PROJ_YIELDS = (DIM // 128) * 2               # 16 per chunk

            # ================= attention chunks =================
            def chunk(b, ihalf, h, pacer, steps_left):
                i0 = b * N + ihalf * 1024
                icol = slice(i0, i0 + 1024)
                hp = slice(h * 64, (h + 1) * 64)
                seg = (b * IH + ihalf) * HPC + h
                po_t = ps.tile(
                    [128, 2, 512], F32, tag="po", name=f"po{seg}"
                )
                ets = {}
                for jt in range(JT + 1):
                    if jt < JT:
                        jcol = slice(b * N + jt * 128, b * N + jt * 128 + 128)
                        st = ps.tile(
                            [128, 2, 512], F32, tag="st", bufs=2,
                            name=f"st{seg}_{jt}",
                        )
                        for a in range(2):
                            nc.tensor.matmul(
                                st[:, a, :], KT[hp, jcol],
                                QT[hp, i0 + a * 512:i0 + (a + 1) * 512],
                                start=True, stop=True,
                            )
                        et = etp.tile([128, 1024], BF16, tag="et", name="et")
                        nc.scalar.activation(
                            et, st.rearrange("p a b -> p (a b)"),
                            mybir.ActivationFunctionType.Exp,
                            scale=SCALE,
                        )
                        ets[jt] = et
                    if jt > 0:
                        jp = jt - 1
                        et = ets.pop(jp)
                        for a in range(2):
                            nc.tensor.matmul(
                                po_t[0:65, a, :],
                                Vt[:, b * JT + jp, h * 65:h * 65 + 65],
                                et[:, a * 512:(a + 1) * 512],
                                start=(jp == 0), stop=(jp == JT - 1),
                            )
                    pacer.drain_paced(steps_left)
                    steps_left -= 1

                # ---- drain: denom row first (longer chain), then o ----
                po_o = po_t.rearrange("p a b -> p (a b)")
                dq = nc.scalar if seg == 15 else nc.sync
                dnst = strm.tile([1, 1024], F32, tag="dnst")
                nc.vector.tensor_copy(dnst, po_o[64:65, :])
                dq.dma_start(out=dn_dram[seg:seg + 1, :], in_=dnst[0:1, :])
                if h == 0:
                    nc.vector.tensor_copy(o_sb[0:64, icol], po_o[0:64, :])
                else:
                    h1s = strm.tile([64, 1024], BF16, tag="h1s")
                    nc.vector.tensor_copy(h1s, po_o[0:64, :])
                    nc.sync.dma_start(out=o_sb[64:128, icol], in_=h1s)
                return steps_left

            def normalize(b, ihalf):
                i0 = b * N + ihalf * 1024
                icol = slice(i0, i0 + 1024)
                g0 = (b * IH + ihalf) * HPC
                dq = nc.scalar if (b == B - 1 and ihalf == IH - 1) else nc.sync
                dns = strm.tile([2, 1024], F32, tag="dns")
                dq.dma_start(out=dns, in_=dn_dram[g0:g0 + 2, :])
                with nc.allow_low_precision(reason="softmax denom approx"):
                    nc.vector.reciprocal_approx_fast(out=dns, in_=dns)
                dq.dma_start(out=rc_dram[g0:g0 + 2, :], in_=dns)
                for h in range(HPC):
                    rows = slice(h * 64, (h + 1) * 64)
                    bcast = strm.tile([128, 1024], F32, tag="bcast")
                    src = rc_dram[g0 + h:g0 + h + 1, :]
                    rbc = bass.AP(
                        tensor=src.tensor,
                        offset=src.offset,
                        ap=[[0, 64]] + list(src.ap)[1:],
                    )
                    dq.dma_start(out=bcast[rows, :], in_=rbc)
                    eng = nc.gpsimd if h == 0 else nc.vector
                    eng.tensor_mul(
                        o_sb[rows, icol], o_sb[rows, icol], bcast[rows, :]
                    )

            # ================= main schedule =================
            # Prologue: batch-0 QKV runs straight (PE ramps to max p-state).
            # x block 0 is the first DMA in the queue; weights follow.
            xin_fetch(0, split=True)
            nc.sync.dma_start(
                out=w_in_sb,
                in_=w_in_c.rearrange("(kt p) m -> p kt m", p=128),
            )
            pro = Pacer()
            pro.add(qkv_gen(0), QKV_YIELDS)
            pro.drain_all()
            nc.sync.dma_start(out=w_out_sb, in_=w_out_c[:, :])

            proj_ready = []
            for b in range(B):
                pacer = Pacer()
                if b < B - 1:
                    pacer.add(qkv_gen(b + 1), QKV_YIELDS)
                if b == B - 1:
                    # deferred output projections fill the last batch
                    for (pb, pih) in proj_ready:
                        pacer.add(proj_gen(pb, pih), PROJ_YIELDS)
                    proj_ready = []
                steps_left = IH * HPC * (JT + 1)   # 68 et-steps in this batch
                for ihalf in range(IH):
                    for h in range(HPC):
                        steps_left = chunk(b, ihalf, h, pacer, steps_left)
                    normalize(b, ihalf)
                    proj_ready.append((b, ihalf))
                if b < B - 1:
                    pacer.drain_all()
            # tail: proj(b3,i0) matmuls cover the last normalize's latency,
            # then proj(b3,i1) finishes
            for (pb, pih) in proj_ready:
                pacer.add(proj_gen(pb, pih), PROJ_YIELDS)
            pacer.drain_all()

    nc.finalize()
    return nc


_CACHED = {}


def kernel(x, w_in, w_out, b_out, _trace=False):
    if "nc" not in _CACHED:
        _CACHED["nc"] = _build_nc()
    nc = _CACHED["nc"]

    x2 = np.ascontiguousarray(
        x.reshape(NT, DIM).T.astype(np.float32)
    )  # [DIM, NT]
    in_maps = []
    for c in range(NCORES):
        h0, h1 = HPC * c, HPC * c + 1
        cols = []
        for part in range(3):  # q, k, v
            base = part * DIM
            cols.extend(range(base + h0 * HD, base + h0 * HD + HD))
            cols.extend(range(base + h1 * HD, base + h1 * HD + HD))
        w_in_cc = np.ascontiguousarray(w_in[:, cols].astype(np.float32))
        w_out_cc = np.ascontiguousarray(
            w_out[128 * c:128 * (c + 1), :].astype(np.float32)
        )
        in_maps.append(
            {
                "xT": x2.astype(ml_dtypes.bfloat16),
                "w_in_c": w_in_cc.astype(ml_dtypes.bfloat16),
                "w_out_c": w_out_cc.astype(ml_dtypes.bfloat16),
            }
        )

    res = run_bass_kernel_spmd(
        nc, in_maps, core_ids=list(range(NCORES)), trace=_trace
    )
    acc = res.results[0]["po"].astype(np.float64)
    for c in range(1, NCORES):
        acc = acc + res.results[c]["po"].astype(np.float64)
    out = acc.T + b_out.astype(np.float64)
    if _trace:
        kernel.last_result = res
    return np.ascontiguousarray(out.reshape(B, N, DIM).astype(np.float32))
